# revision 1
# baseline (speedup 1.0000x reference)
"""NoisyNet dense layer (training mode) on 8 TRN2 NeuronCores.

out[b,u] = x @ W_mu + eps_out * ((x*eps_in) @ W_sigma) + bias_mu + bias_sigma*eps_out

Sharding: data-parallel over batch (4096 -> 512 rows/core), weights/biases
replicated. On-device math runs in a transposed layout ([D,B]/[U,B]) so the
contraction dim D lands on SBUF partitions; the host does the (free)
transposes, bf16 casts and the final gather.
"""

import numpy as np
import ml_dtypes

import concourse.bacc as bacc
import concourse.mybir as mybir
import concourse.tile as tile
from concourse.bass_utils import run_bass_kernel_spmd

N_CORES = 8
B, D, U = 4096, 2048, 2048
BL = B // N_CORES          # 512 batch rows per core
P = 128                    # partitions
KT = D // P                # 16 contraction tiles
UT = U // P                # 16 output tiles
BF16 = mybir.dt.bfloat16
FP32 = mybir.dt.float32

_NBF = ml_dtypes.bfloat16

_cached = None


def _build():
    nc = bacc.Bacc("TRN2", target_bir_lowering=False, debug=False)

    xT = nc.declare_dram_parameter("xT", [KT, P, BL], BF16, isOutput=False)
    eiT = nc.declare_dram_parameter("eiT", [KT, P, BL], BF16, isOutput=False)
    eoT = nc.declare_dram_parameter("eoT", [UT, P, BL], BF16, isOutput=False)
    wmu = nc.declare_dram_parameter("wmu", [UT, P, KT * P], BF16, isOutput=False)
    wsg = nc.declare_dram_parameter("wsg", [UT, P, KT * P], BF16, isOutput=False)
    bmu = nc.declare_dram_parameter("bmu", [P, UT], FP32, isOutput=False)
    bsg = nc.declare_dram_parameter("bsg", [P, UT], FP32, isOutput=False)
    outT = nc.declare_dram_parameter("outT", [UT, P, BL], FP32, isOutput=True)

    with tile.TileContext(nc) as tc:
        with (
            tc.tile_pool(name="acts", bufs=1) as acts,
            tc.tile_pool(name="ei", bufs=3) as eip,
            tc.tile_pool(name="w", bufs=3) as wp,
            tc.tile_pool(name="eo", bufs=3) as eop,
            tc.tile_pool(name="bias", bufs=1) as bp,
            tc.tile_pool(name="psum", bufs=2, space="PSUM") as pp,
            tc.tile_pool(name="tmp", bufs=2) as tp,
            tc.tile_pool(name="out", bufs=3) as op,
        ):
            bmu_t = bp.tile([P, UT], FP32, tag="bmu")
            nc.sync.dma_start(bmu_t[:], bmu[:])
            bsg_t = bp.tile([P, UT], FP32, tag="bsg")
            nc.sync.dma_start(bsg_t[:], bsg[:])

            x_tiles = []
            z_tiles = []
            for k in range(KT):
                xt = acts.tile([P, BL], BF16, tag=f"x{k}")
                nc.sync.dma_start(xt[:], xT[k])
                ei_t = eip.tile([P, BL], BF16, tag="ei")
                nc.sync.dma_start(ei_t[:], eiT[k])
                zt = acts.tile([P, BL], BF16, tag=f"z{k}")
                nc.vector.tensor_mul(zt[:], xt[:], ei_t[:])
                x_tiles.append(xt)
                z_tiles.append(zt)

            for u in range(UT):
                wm = wp.tile([P, KT * P], BF16, tag="wm")
                nc.sync.dma_start(wm[:], wmu[u])
                ws = wp.tile([P, KT * P], BF16, tag="ws")
                nc.sync.dma_start(ws[:], wsg[u])
                eo_t = eop.tile([P, BL], BF16, tag="eo")
                nc.sync.dma_start(eo_t[:], eoT[u])

                ps_m = pp.tile([P, BL], FP32, tag="psm")
                ps_n = pp.tile([P, BL], FP32, tag="psn")
                for k in range(KT):
                    nc.tensor.matmul(
                        ps_m[:],
                        wm[:, k * P:(k + 1) * P],
                        x_tiles[k][:],
                        start=(k == 0),
                        stop=(k == KT - 1),
                    )
                for k in range(KT):
                    nc.tensor.matmul(
                        ps_n[:],
                        ws[:, k * P:(k + 1) * P],
                        z_tiles[k][:],
                        start=(k == 0),
                        stop=(k == KT - 1),
                    )

                t_n = tp.tile([P, BL], FP32, tag="tn")
                nc.scalar.add(t_n[:], ps_n[:], bsg_t[:, u:u + 1])
                t_m = tp.tile([P, BL], FP32, tag="tm")
                nc.scalar.add(t_m[:], ps_m[:], bmu_t[:, u:u + 1])
                pr = tp.tile([P, BL], FP32, tag="pr")
                nc.vector.tensor_mul(pr[:], t_n[:], eo_t[:])
                o = op.tile([P, BL], FP32, tag="o")
                nc.vector.tensor_add(o[:], pr[:], t_m[:])
                nc.sync.dma_start(outT[u], o[:])

    nc.compile()
    return nc


def _get_nc():
    global _cached
    if _cached is None:
        _cached = _build()
    return _cached


def kernel(x, weight_mu, weight_sigma, bias_mu, bias_sigma, eps_in, eps_out,
           _trace=False):
    nc = _get_nc()

    # Host-side layout prep (transposes + bf16 casts only; no layer math).
    def to_kpb(a):  # [B, D] -> per-core [KT, P, BL] (d on partitions)
        a = np.ascontiguousarray(a.astype(_NBF))
        return [
            np.ascontiguousarray(
                a[c * BL:(c + 1) * BL].T.reshape(KT, P, BL))
            for c in range(N_CORES)
        ]

    xs = to_kpb(x)
    eis = to_kpb(eps_in)
    eos = to_kpb(eps_out)  # same transform, u on partitions

    def w_blocks(w):  # [D, U] -> [UT, P(d within block), KT*P] bf16
        wb = w.astype(_NBF).reshape(KT, P, UT, P).transpose(2, 1, 0, 3)
        return np.ascontiguousarray(wb.reshape(UT, P, KT * P))

    wmu_h = w_blocks(weight_mu)
    wsg_h = w_blocks(weight_sigma)
    bmu_h = np.ascontiguousarray(bias_mu.astype(np.float32).reshape(UT, P).T)
    bsg_h = np.ascontiguousarray(bias_sigma.astype(np.float32).reshape(UT, P).T)

    in_maps = [
        {
            "xT": xs[c],
            "eiT": eis[c],
            "eoT": eos[c],
            "wmu": wmu_h,
            "wsg": wsg_h,
            "bmu": bmu_h,
            "bsg": bsg_h,
        }
        for c in range(N_CORES)
    ]

    res = run_bass_kernel_spmd(nc, in_maps, core_ids=list(range(N_CORES)),
                               trace=_trace)
    kernel.last_result = res

    out = np.empty((B, U), dtype=np.float32)
    for c in range(N_CORES):
        oc = res.results[c]["outT"]  # [UT, P, BL]
        out[c * BL:(c + 1) * BL] = oc.transpose(2, 0, 1).reshape(BL, U)
    return out


# revision 2
# speedup vs baseline: 1.0903x; 1.0903x over previous
"""NoisyNet dense layer (training mode) on 8 TRN2 NeuronCores.

out[b,u] = x @ W_mu + eps_out * ((x*eps_in) @ W_sigma) + bias_mu + bias_sigma*eps_out

Sharding: data-parallel over batch (4096 -> 512 rows/core), weights/biases
replicated. On-device math runs in a transposed layout ([D,B]/[U,B]) so the
contraction dim D lands on SBUF partitions; the host does the (free)
transposes, bf16 casts and the final gather.

Schedule: weight chunk for u=0 is DMA'd first (sync queue) in parallel with
chunked x/eps_in loads (scalar queue) so the PE starts within a few us.
"""

import numpy as np
import ml_dtypes

import concourse.bacc as bacc
import concourse.mybir as mybir
import concourse.tile as tile
from concourse.bass_utils import run_bass_kernel_spmd

N_CORES = 8
B, D, U = 4096, 2048, 2048
BL = B // N_CORES          # 512 batch rows per core
P = 128                    # partitions
KT = D // P                # 16 contraction tiles
UT = U // P                # 16 output tiles
KC = 4                     # k-tiles per activation DMA chunk
NCH = KT // KC             # 4 chunks
BF16 = mybir.dt.bfloat16
FP32 = mybir.dt.float32

_NBF = ml_dtypes.bfloat16

_cached = None


def _build():
    nc = bacc.Bacc("TRN2", target_bir_lowering=False, debug=False)

    # activations laid out [P, KT, BL]: partition p holds d = k*128+p
    xT = nc.declare_dram_parameter("xT", [P, KT, BL], BF16, isOutput=False)
    eiT = nc.declare_dram_parameter("eiT", [P, KT, BL], BF16, isOutput=False)
    eoT = nc.declare_dram_parameter("eoT", [P, UT, BL], BF16, isOutput=False)
    wmu = nc.declare_dram_parameter("wmu", [UT, P, KT * P], BF16, isOutput=False)
    wsg = nc.declare_dram_parameter("wsg", [UT, P, KT * P], BF16, isOutput=False)
    bmu = nc.declare_dram_parameter("bmu", [P, UT], FP32, isOutput=False)
    bsg = nc.declare_dram_parameter("bsg", [P, UT], FP32, isOutput=False)
    outT = nc.declare_dram_parameter("outT", [UT, P, BL], FP32, isOutput=True)

    with tile.TileContext(nc) as tc:
        with (
            tc.tile_pool(name="acts", bufs=1) as acts,
            tc.tile_pool(name="w", bufs=3) as wp,
            tc.tile_pool(name="bias", bufs=1) as bp,
            tc.tile_pool(name="psum", bufs=3, space="PSUM") as pp,
            tc.tile_pool(name="tmp", bufs=2) as tp,
            tc.tile_pool(name="out", bufs=3) as op,
        ):
            # Weight chunk for u=0 goes out first on the sync queue.
            w_tiles = {}
            for u in range(2):
                wm = wp.tile([P, KT * P], BF16, tag="wm")
                nc.sync.dma_start(wm[:], wmu[u])
                ws = wp.tile([P, KT * P], BF16, tag="ws")
                nc.sync.dma_start(ws[:], wsg[u])
                w_tiles[u] = (wm, ws)

            # Activations stream on the scalar queue, chunked so the first
            # matmuls can start before the full load finishes.
            x_sb = acts.tile([P, KT, BL], BF16, tag="x")
            ei_sb = acts.tile([P, KT, BL], BF16, tag="ei")
            z_sb = acts.tile([P, KT, BL], BF16, tag="z")
            eo_sb = acts.tile([P, UT, BL], BF16, tag="eo")
            for c in range(NCH):
                s = slice(c * KC, (c + 1) * KC)
                nc.scalar.dma_start(x_sb[:, s, :], xT[:, s, :])
                nc.scalar.dma_start(ei_sb[:, s, :], eiT[:, s, :])
                nc.vector.tensor_mul(z_sb[:, s, :], x_sb[:, s, :], ei_sb[:, s, :])
            bmu_t = bp.tile([P, UT], FP32, tag="bmu")
            nc.scalar.dma_start(bmu_t[:], bmu[:])
            bsg_t = bp.tile([P, UT], FP32, tag="bsg")
            nc.scalar.dma_start(bsg_t[:], bsg[:])
            for c in range(NCH):
                s = slice(c * KC, (c + 1) * KC)
                nc.scalar.dma_start(eo_sb[:, s, :], eoT[:, s, :])

            for u in range(UT):
                if u in w_tiles:
                    wm, ws = w_tiles.pop(u)
                else:
                    wm = wp.tile([P, KT * P], BF16, tag="wm")
                    nc.sync.dma_start(wm[:], wmu[u])
                    ws = wp.tile([P, KT * P], BF16, tag="ws")
                    nc.sync.dma_start(ws[:], wsg[u])

                ps_m = pp.tile([P, BL], FP32, tag="psm")
                ps_n = pp.tile([P, BL], FP32, tag="psn")
                for k in range(KT):
                    nc.tensor.matmul(
                        ps_m[:],
                        wm[:, k * P:(k + 1) * P],
                        x_sb[:, k, :],
                        start=(k == 0),
                        stop=(k == KT - 1),
                    )
                for k in range(KT):
                    nc.tensor.matmul(
                        ps_n[:],
                        ws[:, k * P:(k + 1) * P],
                        z_sb[:, k, :],
                        start=(k == 0),
                        stop=(k == KT - 1),
                    )

                t_n = tp.tile([P, BL], FP32, tag="tn")
                nc.scalar.add(t_n[:], ps_n[:], bsg_t[:, u:u + 1])
                t_m = tp.tile([P, BL], FP32, tag="tm")
                nc.scalar.add(t_m[:], ps_m[:], bmu_t[:, u:u + 1])
                pr = tp.tile([P, BL], FP32, tag="pr")
                nc.vector.tensor_mul(pr[:], t_n[:], eo_sb[:, u, :])
                o = op.tile([P, BL], FP32, tag="o")
                nc.vector.tensor_add(o[:], pr[:], t_m[:])
                nc.scalar.dma_start(outT[u], o[:])

    nc.compile()
    return nc


def _get_nc():
    global _cached
    if _cached is None:
        _cached = _build()
    return _cached


def kernel(x, weight_mu, weight_sigma, bias_mu, bias_sigma, eps_in, eps_out,
           _trace=False):
    nc = _get_nc()

    # Host-side layout prep (transposes + bf16 casts only; no layer math).
    def to_pkb(a):  # [B, D] -> per-core [P, KT, BL] (partition p holds k*128+p)
        a = np.ascontiguousarray(a.astype(_NBF))
        return [
            np.ascontiguousarray(
                a[c * BL:(c + 1) * BL].T.reshape(KT, P, BL).transpose(1, 0, 2))
            for c in range(N_CORES)
        ]

    xs = to_pkb(x)
    eis = to_pkb(eps_in)
    eos = to_pkb(eps_out)  # same transform, u in place of k

    def w_blocks(w):  # [D, U] -> [UT, P(d within block), KT*P] bf16
        wb = w.astype(_NBF).reshape(KT, P, UT, P).transpose(2, 1, 0, 3)
        return np.ascontiguousarray(wb.reshape(UT, P, KT * P))

    wmu_h = w_blocks(weight_mu)
    wsg_h = w_blocks(weight_sigma)
    bmu_h = np.ascontiguousarray(bias_mu.astype(np.float32).reshape(UT, P).T)
    bsg_h = np.ascontiguousarray(bias_sigma.astype(np.float32).reshape(UT, P).T)

    in_maps = [
        {
            "xT": xs[c],
            "eiT": eis[c],
            "eoT": eos[c],
            "wmu": wmu_h,
            "wsg": wsg_h,
            "bmu": bmu_h,
            "bsg": bsg_h,
        }
        for c in range(N_CORES)
    ]

    res = run_bass_kernel_spmd(nc, in_maps, core_ids=list(range(N_CORES)),
                               trace=_trace)
    kernel.last_result = res

    out = np.empty((B, U), dtype=np.float32)
    for c in range(N_CORES):
        oc = res.results[c]["outT"]  # [UT, P, BL]
        out[c * BL:(c + 1) * BL] = oc.transpose(2, 0, 1).reshape(BL, U)
    return out
